# revision 5
# baseline (speedup 1.0000x reference)
"""Causal self-attention Trainium2 kernel (B=8, S=1024, C=768, H=12).

Sharding: pure data-parallel over batch — core i computes batch i end-to-end.
No collectives. Weights are replicated to all 8 cores.

Software-pipelined schedule (v2): attention for q-block b runs interleaved
with projection work for later blocks and out-projection for earlier blocks,
so the Tensor engine never idles waiting on the Activation engine's exp.
Head PAIRS share one [128, 1024] 2-bank PSUM logits tile so a single exp
activation covers both heads (halves Act-engine instruction overhead).

Per-core math (batch b):
  xT        [C, S]   (host-transposed slice of x)
  Q,K       [c'=h*64+d, S] layout  (projection with feature dim on partitions)
  V(+ones)  [S, h, 65] layout      (natural layout + fused ones column)
  logits    [s_k, s_q] (transposed) -> exp on ScalarE -> P
  AV        psum[65, s_q] = [V_h | 1]^T P   (row 64 = softmax denominator)
  y         [c, S] layout, normalized via DMA-broadcast reciprocal row
  out       [S, C] via out-proj with y tiles as the stationary operand
"""

import sys
import types
from collections import deque

import numpy as np

import concourse.bass as bass
import concourse.mybir as mybir
import concourse.tile as tile
from concourse import bacc
from concourse.masks import make_upper_triangular


def _ensure_axon_hooks():
    """The container's `antenv` stub lacks `axon_hooks`, which
    run_bass_kernel_spmd imports when trace=True under axon. Provide it and
    register the NTFF profile hook so tracing works."""
    try:
        import antenv.axon_hooks  # noqa: F401

        return
    except ImportError:
        pass
    try:
        import antenv
    except ImportError:
        return
    mod = types.ModuleType("antenv.axon_hooks")
    _store = [None]
    mod.set_axon_ntff_profile_hook = lambda h: _store.__setitem__(0, h)
    mod.get_axon_ntff_profile_hook = lambda: _store[0]
    sys.modules["antenv.axon_hooks"] = mod
    antenv.axon_hooks = mod
    try:
        from trn_agent_boot.trn_boot import _ntff_profile_via_ctypes

        hook = _ntff_profile_via_ctypes("/opt/axon/libaxon_pjrt.so")
        mod.set_axon_ntff_profile_hook(hook)
    except Exception:
        pass


_ensure_axon_hooks()

P = 128
C = 768
H = 12
D = 64
NT_C = C // P          # 6 c-tiles
QB = 256               # q-block (matmul moving free dim)
F32 = mybir.dt.float32
F16 = mybir.dt.float16


def build_nc(S=1024):
    NT_S = S // P          # 8 s-tiles
    NB = S // QB           # 4 q-blocks

    nc = bacc.Bacc("TRN2", target_bir_lowering=False, debug=False)

    xt_d = nc.dram_tensor("xt", [C, S], F16, kind="ExternalInput")
    # wqkTt[t, p, ct*128+n] = wqkT[ct*128+p, t*128+n]: per-(t) tiles with
    # contiguous per-partition lines for efficient DMA.
    wqk_d = nc.dram_tensor("wqkTt", [2 * NT_C, P, C], F16, kind="ExternalInput")
    wv_d = nc.dram_tensor("wvT", [C, C], F16, kind="ExternalInput")
    wo_d = nc.dram_tensor("woutT", [C, C], F16, kind="ExternalInput")
    bqk_d = nc.dram_tensor("bqk", [2 * C], F32, kind="ExternalInput")
    bv_d = nc.dram_tensor("bv", [C], F32, kind="ExternalInput")
    bo_d = nc.dram_tensor("bout", [C], F32, kind="ExternalInput")
    out_d = nc.dram_tensor("out", [S, C], F32, kind="ExternalOutput")

    with tile.TileContext(nc) as tc:
        with (
            tc.tile_pool(name="const", bufs=1) as cpool,
            tc.tile_pool(name="big", bufs=1) as gpool,
            tc.tile_pool(name="ptile", bufs=4) as ppool,
            tc.tile_pool(name="evac", bufs=3) as epool,
            tc.tile_pool(name="recip", bufs=4) as rcpool,
            tc.tile_pool(name="bcast", bufs=4) as bpool,
            tc.tile_pool(name="proj_ps", bufs=2, space="PSUM") as proj_ps,
            tc.tile_pool(name="logit_ps", bufs=2, space="PSUM") as logit_ps,
            tc.tile_pool(name="av_ps", bufs=2, space="PSUM") as av_ps,
        ):
            # ---------------- constants ----------------
            trimask = cpool.tile([P, P], F16)      # 1.0 where p <= f else 0.0
            make_upper_triangular(nc, trimask[:], val=1.0, diag=True)
            trimask_r = trimask[:]

            bqk_sb = cpool.tile([P, 2 * NT_C], F32)
            nc.scalar.dma_start(bqk_sb[:], bqk_d[:].rearrange("(t p) -> p t", p=P))
            bv_bc = cpool.tile([P, C], F32)
            nc.scalar.dma_start(bv_bc[:], bv_d[:][None, :].to_broadcast((P, C)))
            bo_bc = cpool.tile([P, C], F32)
            nc.scalar.dma_start(bo_bc[:], bo_d[:][None, :].to_broadcast((P, C)))

            # ---------------- persistent SBUF tensors ----------------
            xt_sb = gpool.tile([P, NT_C, S], F16)
            qk_sb = gpool.tile([P, 2 * NT_C, S], F16)   # Q tiles 0..5, K 6..11
            vp_sb = gpool.tile([P, NT_S, H, D + 1], F16)  # [s, st, h, d|1]
            nc.vector.memset(vp_sb[:, :, :, D : D + 1], 1.0)
            y_sb = gpool.tile([P, NT_C, S], F16)

            wqk_sb = gpool.tile([P, 2 * NT_C, C], F16)  # [p, t, ct*128+n]
            wv_sb = gpool.tile([P, NT_C, C], F16)
            wo_sb = gpool.tile([P, NT_C, C], F16)

            xt_r = xt_d[:, :].rearrange("(ct p) s -> p ct s", p=P)
            wv_r = wv_d[:, :].rearrange("(ct p) n -> p ct n", p=P)
            wo_r = wo_d[:, :].rearrange("(ct p) n -> p ct n", p=P)

            # ---------------- input DMA schedule ----------------
            # sync queue: xt first half, wqk tiles (in consumption order),
            #             xt second half.
            HS = S // 2
            for ct in range(NT_C):
                nc.sync.dma_start(xt_sb[:, ct, 0:HS], xt_r[:, ct, 0:HS])
            for hp in range(NT_C):
                for t in (hp, NT_C + hp):
                    nc.sync.dma_start(wqk_sb[:, t, :], wqk_d[t, :, :])
            for ct in range(NT_C):
                nc.sync.dma_start(xt_sb[:, ct, HS:S], xt_r[:, ct, HS:S])
            # gpsimd (software DGE) queue: wv chunks then wo; the per-head
            # reciprocal broadcasts interleave after these in program order.
            for ct in range(NT_C):
                nc.gpsimd.dma_start(wv_sb[:, ct, 0:512], wv_r[:, ct, 0:512])
            for ct in range(NT_C):
                nc.gpsimd.dma_start(wv_sb[:, ct, 512:C], wv_r[:, ct, 512:C])
            for ct in range(NT_C):
                nc.gpsimd.dma_start(wo_sb[:, ct, :], wo_r[:, ct, :])

            # ---------------- work-item builders ----------------
            def qk_group(t, half):
                s0 = half * HS
                ps = proj_ps.tile([P, HS], F32, tag="proj")
                for ct in range(NT_C):
                    nc.tensor.matmul(
                        ps[:],
                        wqk_sb[:, t, ct * P : (ct + 1) * P],
                        xt_sb[:, ct, s0 : s0 + HS],
                        start=(ct == 0),
                        stop=(ct == NT_C - 1),
                    )
                nc.vector.tensor_scalar_add(
                    qk_sb[:, t, s0 : s0 + HS], ps[:], bqk_sb[:, t : t + 1]
                )

            def v_group(st, ci):
                cs, cw = (0, 512) if ci == 0 else (512, 256)
                ps = proj_ps.tile([P, HS], F32, tag="proj")
                for ct in range(NT_C):
                    nc.tensor.matmul(
                        ps[:, :cw],
                        xt_sb[:, ct, st * P : (st + 1) * P],
                        wv_sb[:, ct, cs : cs + cw],
                        start=(ct == 0),
                        stop=(ct == NT_C - 1),
                    )
                nh = cw // D
                h0 = cs // D
                nc.vector.tensor_add(
                    vp_sb[:, st, h0 : h0 + nh, 0:D],
                    ps[:, :cw].rearrange("p (h d) -> p h d", d=D),
                    bv_bc[:, cs : cs + cw].rearrange("p (h d) -> p h d", d=D),
                )

            ot_tiles = {}

            def outp_group(st, ci):
                cs, cw = (0, 512) if ci == 0 else (512, 256)
                if ci == 0:
                    ot_tiles[st] = epool.tile([P, C], F32, tag="ot", name=f"ot_{st}")
                ot = ot_tiles[st]
                ps = proj_ps.tile([P, HS], F32, tag="proj")
                for ct in range(NT_C):
                    nc.tensor.matmul(
                        ps[:, :cw],
                        y_sb[:, ct, st * P : (st + 1) * P],
                        wo_sb[:, ct, cs : cs + cw],
                        start=(ct == 0),
                        stop=(ct == NT_C - 1),
                    )
                nc.vector.tensor_add(
                    ot[:, cs : cs + cw], ps[:, :cw], bo_bc[:, cs : cs + cw]
                )
                if ci == 1:
                    nc.sync.dma_start(out_d[st * P : (st + 1) * P, :], ot[:])

            # ---------------- attention (per head-pair) ----------------
            pending = deque()  # deferred y-normalization multiplies

            def flush_pending():
                while pending:
                    pending.popleft()()

            def attn_pair(b, hp, drain=None):
                flush_pending()
                kt = NT_C + hp
                avs = [
                    av_ps.tile([D + 1, QB], F32, tag="av", name=f"av_{b}_{hp}_{hh}")
                    for hh in (0, 1)
                ]
                pts = []

                def av_mms(jp):
                    pt2 = pts[jp]
                    for hh in (0, 1):
                        h = 2 * hp + hh
                        for dj in (0, 1):
                            j = 2 * jp + dj
                            nc.tensor.matmul(
                                avs[hh][:],
                                vp_sb[:, j, h, :],
                                pt2[:, hh * 2 * QB + dj * QB : hh * 2 * QB + (dj + 1) * QB],
                                start=(j == 0),
                                stop=(j == 2 * b + 1),
                            )

                for jp in range(b + 1):
                    lg2 = logit_ps.tile([P, 4 * QB], F32, tag="lg")
                    for hh in (0, 1):
                        lo = hh * D
                        for dj in (0, 1):
                            j = 2 * jp + dj
                            nc.tensor.matmul(
                                lg2[:, hh * 2 * QB + dj * QB : hh * 2 * QB + (dj + 1) * QB],
                                qk_sb[lo : lo + D, kt, j * P : (j + 1) * P],
                                qk_sb[lo : lo + D, hp, b * QB : (b + 1) * QB],
                                start=True,
                                stop=True,
                                skip_group_check=True,
                            )
                    pt2 = ppool.tile([P, 4 * QB], F16, tag="pt")
                    nc.scalar.activation(
                        pt2[:], lg2[:],
                        mybir.ActivationFunctionType.Exp, scale=0.125,
                    )
                    if jp == b:  # diagonal pair: causal masking
                        for hh in (0, 1):
                            base = hh * 2 * QB
                            nc.vector.tensor_mul(
                                pt2[:, base : base + P],
                                pt2[:, base : base + P],
                                trimask_r,
                            )
                            nc.vector.memset(pt2[:, base + QB : base + QB + P], 0.0)
                            nc.vector.tensor_mul(
                                pt2[:, base + QB + P : base + 2 * QB],
                                pt2[:, base + QB + P : base + 2 * QB],
                                trimask_r,
                            )
                    pts.append(pt2)
                    if jp >= 1:
                        av_mms(jp - 1)
                    if drain is not None:
                        drain.step()
                av_mms(b)
                # per-head normalization: reciprocal of denominator row,
                # DMA-broadcast across 64 partitions, deferred multiply-evac.
                for hh in (0, 1):
                    h = 2 * hp + hh
                    rc = rcpool.tile([1, QB], F16, tag="rc", name=f"rc_{b}_{h}")
                    with nc.allow_low_precision(
                        reason="fp16 reciprocal of softmax denominators"
                    ):
                        nc.vector.reciprocal(rc[:], avs[hh][D : D + 1, :])
                    bc = bpool.tile([D, QB], F16, tag="bc", name=f"bc_{b}_{h}")
                    nc.gpsimd.partition_broadcast(bc[:], rc[:])
                    lo2 = hh * D

                    def _norm(av=avs[hh], bc=bc, lo2=lo2, hp=hp, b=b):
                        nc.vector.tensor_mul(
                            y_sb[lo2 : lo2 + D, hp, b * QB : (b + 1) * QB],
                            av[0:D, :],
                            bc[:],
                        )

                    pending.append(_norm)

            # ---------------- filler drain ----------------
            class Drainer:
                def __init__(self):
                    self.items = []
                    self.acc = 0.0
                    self.rate = 0.0

                def load(self, items, units):
                    self.items = list(items)
                    self.acc = 0.0
                    self.rate = len(self.items) / max(units, 1)

                def step(self):
                    self.acc += self.rate
                    while self.items and self.acc >= 1.0:
                        self.items.pop(0)()
                        self.acc -= 1.0

                def flush(self):
                    for f in self.items:
                        f()
                    self.items = []

            drain = Drainer()

            # ---------------- prologue: proj(first half) + attn block 0 ----
            qk_group(0, 0)
            qk_group(NT_C + 0, 0)
            v_group(0, 0)
            v_group(1, 0)
            attn_pair(0, 0)
            for hp in range(1, NT_C):
                if hp == 4:
                    v_group(0, 1)
                    v_group(1, 1)
                qk_group(hp, 0)
                qk_group(NT_C + hp, 0)
                attn_pair(0, hp)

            # ---------------- stages b = 1..3 ----------------
            fillers = {
                1: [lambda t=t: qk_group(t, 1) for t in range(2 * NT_C)]
                + [lambda st=st, ci=ci: outp_group(st, ci)
                   for st in (0, 1) for ci in (0, 1)],
                2: [lambda st=st, ci=ci: outp_group(st, ci)
                    for st in (2, 3) for ci in (0, 1)],
                3: [lambda st=st, ci=ci: outp_group(st, ci)
                    for st in (4, 5) for ci in (0, 1)],
            }
            for b in range(1, NB):
                for st in (2 * b, 2 * b + 1):
                    for ci in (0, 1):
                        v_group(st, ci)
                drain.load(fillers[b], units=NT_C * (b + 1))
                for hp in range(NT_C):
                    attn_pair(b, hp, drain)
                drain.flush()
            flush_pending()
            for st in (6, 7):
                for ci in (0, 1):
                    outp_group(st, ci)

    nc.compile()
    return nc


_NC_CACHE = {}


def _get_nc(S):
    if S not in _NC_CACHE:
        _NC_CACHE[S] = build_nc(S)
    return _NC_CACHE[S]


def make_in_maps(x, w_qkv, b_qkv, w_out, b_out):
    x = np.asarray(x, np.float32)
    w_qkv = np.asarray(w_qkv, np.float32)
    b_qkv = np.asarray(b_qkv, np.float32)
    w_out = np.asarray(w_out, np.float32)
    b_out = np.asarray(b_out, np.float32)
    B = x.shape[0]
    xt = np.ascontiguousarray(x.transpose(0, 2, 1)).astype(np.float16)
    wqkT = w_qkv[: 2 * C].T.astype(np.float16)          # [C, 2C]
    # [2C? -> t, p, ct, n] tiled layout: wqkTt[t, p, ct*128+n]
    wqkTt = np.ascontiguousarray(
        wqkT.reshape(NT_C, P, 2 * NT_C, P).transpose(2, 1, 0, 3).reshape(
            2 * NT_C, P, C
        )
    )
    wvT = np.ascontiguousarray(w_qkv[2 * C :].T).astype(np.float16)
    woT = np.ascontiguousarray(w_out.T).astype(np.float16)
    bqk = np.ascontiguousarray(b_qkv[: 2 * C])
    bv = np.ascontiguousarray(b_qkv[2 * C :])
    bo = np.ascontiguousarray(b_out)
    return [
        {
            "xt": xt[i],
            "wqkTt": wqkTt,
            "wvT": wvT,
            "woutT": woT,
            "bqk": bqk,
            "bv": bv,
            "bout": bo,
        }
        for i in range(B)
    ]


def kernel_with_results(x, w_qkv, b_qkv, w_out, b_out, attention_mask=None, **run_kw):
    from concourse.bass_utils import run_bass_kernel_spmd

    B, S, C_ = x.shape
    assert C_ == C
    nc = _get_nc(S)
    in_maps = make_in_maps(x, w_qkv, b_qkv, w_out, b_out)
    res = run_bass_kernel_spmd(nc, in_maps, core_ids=list(range(B)), **run_kw)
    out = np.stack([m["out"] for m in res.results], axis=0).astype(np.float32)
    return out, res


def kernel(x, w_qkv, b_qkv, w_out, b_out, attention_mask=None):
    out, _ = kernel_with_results(x, w_qkv, b_qkv, w_out, b_out, attention_mask)
    return out


# revision 10
# speedup vs baseline: 1.0669x; 1.0669x over previous
"""Causal self-attention Trainium2 kernel (B=8, S=1024, C=768, H=12).

Sharding: pure data-parallel over batch — core i computes batch i end-to-end.
No collectives. Weights are replicated to all 8 cores.

Software-pipelined schedule (v2): attention for q-block b runs interleaved
with projection work for later blocks and out-projection for earlier blocks,
so the Tensor engine never idles waiting on the Activation engine's exp.
Head PAIRS share one [128, 1024] 2-bank PSUM logits tile so a single exp
activation covers both heads (halves Act-engine instruction overhead).

Per-core math (batch b):
  xT        [C, S]   (host-transposed slice of x)
  Q,K       [c'=h*64+d, S] layout  (projection with feature dim on partitions)
  V(+ones)  [S, h, 65] layout      (natural layout + fused ones column)
  logits    [s_k, s_q] (transposed) -> exp on ScalarE -> P
  AV        psum[65, s_q] = [V_h | 1]^T P   (row 64 = softmax denominator)
  y         [c, S] layout, normalized via DMA-broadcast reciprocal row
  out       [S, C] via out-proj with y tiles as the stationary operand
"""

import sys
import types
from collections import deque

import numpy as np

import concourse.bass as bass
import concourse.mybir as mybir
import concourse.tile as tile
from concourse import bacc
from concourse.masks import make_upper_triangular


def _ensure_axon_hooks():
    """The container's `antenv` stub lacks `axon_hooks`, which
    run_bass_kernel_spmd imports when trace=True under axon. Provide it and
    register the NTFF profile hook so tracing works."""
    try:
        import antenv.axon_hooks  # noqa: F401

        return
    except ImportError:
        pass
    try:
        import antenv
    except ImportError:
        return
    mod = types.ModuleType("antenv.axon_hooks")
    _store = [None]
    mod.set_axon_ntff_profile_hook = lambda h: _store.__setitem__(0, h)
    mod.get_axon_ntff_profile_hook = lambda: _store[0]
    sys.modules["antenv.axon_hooks"] = mod
    antenv.axon_hooks = mod
    try:
        from trn_agent_boot.trn_boot import _ntff_profile_via_ctypes

        hook = _ntff_profile_via_ctypes("/opt/axon/libaxon_pjrt.so")
        mod.set_axon_ntff_profile_hook(hook)
    except Exception:
        pass


_ensure_axon_hooks()

P = 128
C = 768
H = 12
D = 64
NT_C = C // P          # 6 c-tiles
QB = 256               # q-block (matmul moving free dim)
F32 = mybir.dt.float32
F16 = mybir.dt.float16


def build_nc(S=1024):
    NT_S = S // P          # 8 s-tiles
    NB = S // QB           # 4 q-blocks

    nc = bacc.Bacc("TRN2", target_bir_lowering=False, debug=False)

    xt_d = nc.dram_tensor("xt", [C, S], F16, kind="ExternalInput")
    # wqkTt[t, p, ct*128+n] = wqkT[ct*128+p, t*128+n]: per-(t) tiles with
    # contiguous per-partition lines for efficient DMA.
    wqk_d = nc.dram_tensor("wqkTt", [2 * NT_C, P, C], F16, kind="ExternalInput")
    wv_d = nc.dram_tensor("wvT", [C, C], F16, kind="ExternalInput")
    wo_d = nc.dram_tensor("woutT", [C, C], F16, kind="ExternalInput")
    bqk_d = nc.dram_tensor("bqk", [2 * C], F32, kind="ExternalInput")
    bv_d = nc.dram_tensor("bv", [C], F32, kind="ExternalInput")
    bo_d = nc.dram_tensor("bout", [C], F32, kind="ExternalInput")
    out_d = nc.dram_tensor("out", [S, C], F32, kind="ExternalOutput")

    with tile.TileContext(nc) as tc:
        with (
            tc.tile_pool(name="const", bufs=1) as cpool,
            tc.tile_pool(name="big", bufs=1) as gpool,
            tc.tile_pool(name="ptile", bufs=4) as ppool,
            tc.tile_pool(name="evac", bufs=3) as epool,
            tc.tile_pool(name="recip", bufs=4) as rcpool,
            tc.tile_pool(name="bcast", bufs=4) as bpool,
            tc.tile_pool(name="proj_ps", bufs=2, space="PSUM") as proj_ps,
            tc.tile_pool(name="logit_ps", bufs=2, space="PSUM") as logit_ps,
            tc.tile_pool(name="av_ps", bufs=2, space="PSUM") as av_ps,
        ):
            # ---------------- constants ----------------
            # Fused diagonal-pair mask over a [s_k-pair, q-block] logits tile:
            # cols 0:128 tri, 128:256 keep, 256:384 zero, 384:512 tri.
            dmask = cpool.tile([P, 2 * QB], F16)
            make_upper_triangular(nc, dmask[:, 0:P], val=1.0, diag=True)
            nc.vector.memset(dmask[:, P:QB], 1.0)
            nc.vector.memset(dmask[:, QB : QB + P], 0.0)
            make_upper_triangular(nc, dmask[:, QB + P : 2 * QB], val=1.0, diag=True)
            dmask_r = dmask[:]

            bqk_sb = cpool.tile([P, 2 * NT_C], F32)
            nc.scalar.dma_start(bqk_sb[:], bqk_d[:].rearrange("(t p) -> p t", p=P))
            bv_bc = cpool.tile([P, C], F32)
            nc.scalar.dma_start(bv_bc[:], bv_d[:][None, :].to_broadcast((P, C)))
            bo_bc = cpool.tile([P, C], F32)
            nc.scalar.dma_start(bo_bc[:], bo_d[:][None, :].to_broadcast((P, C)))

            # ---------------- persistent SBUF tensors ----------------
            xt_sb = gpool.tile([P, NT_C, S], F16)
            qk_sb = gpool.tile([P, 2 * NT_C, S], F16)   # Q tiles 0..5, K 6..11
            vp_sb = gpool.tile([P, NT_S, H, D + 1], F16)  # [s, st, h, d|1]
            nc.vector.memset(vp_sb[:, :, :, D : D + 1], 1.0)
            y_sb = gpool.tile([P, NT_C, S], F16)

            wqk_sb = gpool.tile([P, 2 * NT_C, C], F16)  # [p, t, ct*128+n]
            wv_sb = gpool.tile([P, NT_C, C], F16)
            wo_sb = gpool.tile([P, NT_C, C], F16)

            xt_r = xt_d[:, :].rearrange("(ct p) s -> p ct s", p=P)
            wv_r = wv_d[:, :].rearrange("(ct p) n -> p ct n", p=P)
            wo_r = wo_d[:, :].rearrange("(ct p) n -> p ct n", p=P)

            # ---------------- input DMA schedule ----------------
            # sync queue: xt first half, wqk tiles (in consumption order),
            #             xt second half.
            HS = S // 2
            for ct in range(NT_C):
                nc.sync.dma_start(xt_sb[:, ct, 0:HS], xt_r[:, ct, 0:HS])
            for hp in range(NT_C):
                for t in (hp, NT_C + hp):
                    nc.sync.dma_start(wqk_sb[:, t, :], wqk_d[t, :, :])
            for ct in range(NT_C):
                nc.sync.dma_start(xt_sb[:, ct, HS:S], xt_r[:, ct, HS:S])
            # gpsimd (software DGE) queue: wv chunks then wo; the per-head
            # reciprocal broadcasts interleave after these in program order.
            for ct in range(NT_C):
                nc.gpsimd.dma_start(wv_sb[:, ct, 0:512], wv_r[:, ct, 0:512])
            for ct in range(NT_C):
                nc.gpsimd.dma_start(wv_sb[:, ct, 512:C], wv_r[:, ct, 512:C])
            for ct in range(NT_C):
                nc.gpsimd.dma_start(wo_sb[:, ct, :], wo_r[:, ct, :])

            # ---------------- work-item builders ----------------
            def qk_group(t, half):
                s0 = half * HS
                ps = proj_ps.tile([P, HS], F32, tag="proj")
                for ct in range(NT_C):
                    nc.tensor.matmul(
                        ps[:],
                        wqk_sb[:, t, ct * P : (ct + 1) * P],
                        xt_sb[:, ct, s0 : s0 + HS],
                        start=(ct == 0),
                        stop=(ct == NT_C - 1),
                    )
                nc.scalar.activation(
                    qk_sb[:, t, s0 : s0 + HS], ps[:],
                    mybir.ActivationFunctionType.Identity,
                    bias=bqk_sb[:, t : t + 1],
                )

            def v_group(st, ci):
                cs, cw = (0, 512) if ci == 0 else (512, 256)
                ps = proj_ps.tile([P, HS], F32, tag="proj")
                for ct in range(NT_C):
                    nc.tensor.matmul(
                        ps[:, :cw],
                        xt_sb[:, ct, st * P : (st + 1) * P],
                        wv_sb[:, ct, cs : cs + cw],
                        start=(ct == 0),
                        stop=(ct == NT_C - 1),
                    )
                nh = cw // D
                h0 = cs // D
                nc.vector.tensor_add(
                    vp_sb[:, st, h0 : h0 + nh, 0:D],
                    ps[:, :cw].rearrange("p (h d) -> p h d", d=D),
                    bv_bc[:, cs : cs + cw].rearrange("p (h d) -> p h d", d=D),
                )

            ot_tiles = {}

            def outp_group(st, ci):
                cs, cw = (0, 512) if ci == 0 else (512, 256)
                if ci == 0:
                    ot_tiles[st] = epool.tile([P, C], F32, tag="ot", name=f"ot_{st}")
                ot = ot_tiles[st]
                ps = proj_ps.tile([P, HS], F32, tag="proj")
                for ct in range(NT_C):
                    nc.tensor.matmul(
                        ps[:, :cw],
                        y_sb[:, ct, st * P : (st + 1) * P],
                        wo_sb[:, ct, cs : cs + cw],
                        start=(ct == 0),
                        stop=(ct == NT_C - 1),
                    )
                nc.vector.tensor_add(
                    ot[:, cs : cs + cw], ps[:, :cw], bo_bc[:, cs : cs + cw]
                )
                if ci == 1:
                    nc.sync.dma_start(out_d[st * P : (st + 1) * P, :], ot[:])

            # ---------------- attention (per head-pair) ----------------
            pending = deque()  # deferred y-normalization multiplies

            def flush_pending():
                while pending:
                    pending.popleft()()

            def attn_pair(b, hp, drain=None):
                flush_pending()
                kt = NT_C + hp
                avs = [
                    av_ps.tile([D + 1, QB], F32, tag="av", name=f"av_{b}_{hp}_{hh}")
                    for hh in (0, 1)
                ]
                pts = []

                def av_mms(jp):
                    pt2 = pts[jp]
                    for hh in (0, 1):
                        h = 2 * hp + hh
                        for dj in (0, 1):
                            j = 2 * jp + dj
                            nc.tensor.matmul(
                                avs[hh][:],
                                vp_sb[:, j, h, :],
                                pt2[:, hh * 2 * QB + dj * QB : hh * 2 * QB + (dj + 1) * QB],
                                start=(j == 0),
                                stop=(j == 2 * b + 1),
                            )

                for jp in range(b + 1):
                    lg2 = logit_ps.tile([P, 4 * QB], F32, tag="lg")
                    for hh in (0, 1):
                        lo = hh * D
                        for dj in (0, 1):
                            j = 2 * jp + dj
                            nc.tensor.matmul(
                                lg2[:, hh * 2 * QB + dj * QB : hh * 2 * QB + (dj + 1) * QB],
                                qk_sb[lo : lo + D, kt, j * P : (j + 1) * P],
                                qk_sb[lo : lo + D, hp, b * QB : (b + 1) * QB],
                                start=True,
                                stop=True,
                                skip_group_check=True,
                            )
                    pt2 = ppool.tile([P, 4 * QB], F16, tag="pt")
                    nc.scalar.activation(
                        pt2[:], lg2[:],
                        mybir.ActivationFunctionType.Exp, scale=0.125,
                    )
                    if jp == b:  # diagonal pair: fused causal masking
                        for hh in (0, 1):
                            base = hh * 2 * QB
                            nc.vector.tensor_mul(
                                pt2[:, base : base + 2 * QB],
                                pt2[:, base : base + 2 * QB],
                                dmask_r,
                            )
                    pts.append(pt2)
                    if jp >= 1:
                        av_mms(jp - 1)
                    if drain is not None:
                        drain.step()
                av_mms(b)
                # per-head normalization: reciprocal of denominator row,
                # DMA-broadcast across 64 partitions, deferred multiply-evac.
                for hh in (0, 1):
                    h = 2 * hp + hh
                    rc = rcpool.tile([1, QB], F32, tag="rc", name=f"rc_{b}_{h}")
                    nc.vector.reciprocal(rc[:], avs[hh][D : D + 1, :])
                    bc = bpool.tile([D, QB], F32, tag="bc", name=f"bc_{b}_{h}")
                    nc.gpsimd.partition_broadcast(bc[:], rc[:])
                    lo2 = hh * D

                    def _norm(av=avs[hh], bc=bc, lo2=lo2, hp=hp, b=b):
                        nc.vector.tensor_mul(
                            y_sb[lo2 : lo2 + D, hp, b * QB : (b + 1) * QB],
                            av[0:D, :],
                            bc[:],
                        )

                    pending.append(_norm)

            # ---------------- filler drain ----------------
            class Drainer:
                def __init__(self):
                    self.items = []
                    self.acc = 0.0
                    self.rate = 0.0

                def load(self, items, units):
                    self.items = list(items)
                    self.acc = 0.0
                    self.rate = len(self.items) / max(units, 1)

                def step(self):
                    self.acc += self.rate
                    while self.items and self.acc >= 1.0:
                        self.items.pop(0)()
                        self.acc -= 1.0

                def flush(self):
                    for f in self.items:
                        f()
                    self.items = []

            drain = Drainer()

            # ---------------- prologue: proj(first half) + attn block 0 ----
            qk_group(0, 0)
            qk_group(NT_C + 0, 0)
            v_group(0, 0)
            v_group(1, 0)
            attn_pair(0, 0)
            for hp in range(1, NT_C):
                if hp == 4:
                    v_group(0, 1)
                    v_group(1, 1)
                qk_group(hp, 0)
                qk_group(NT_C + hp, 0)
                attn_pair(0, hp)

            # ---------------- stages b = 1..3 ----------------
            fillers = {
                1: [lambda t=t: qk_group(t, 1) for t in range(2 * NT_C)]
                + [lambda st=st, ci=ci: outp_group(st, ci)
                   for st in (0, 1) for ci in (0, 1)],
                2: [lambda st=st, ci=ci: outp_group(st, ci)
                    for st in (2, 3) for ci in (0, 1)],
                3: [lambda st=st, ci=ci: outp_group(st, ci)
                    for st in (4, 5) for ci in (0, 1)],
            }
            for b in range(1, NB):
                for st in (2 * b, 2 * b + 1):
                    for ci in (0, 1):
                        v_group(st, ci)
                drain.load(fillers[b], units=NT_C * (b + 1))
                for hp in range(NT_C):
                    attn_pair(b, hp, drain)
                drain.flush()
            flush_pending()
            for st in (6, 7):
                for ci in (0, 1):
                    outp_group(st, ci)

    nc.compile()
    return nc


_NC_CACHE = {}


def _get_nc(S):
    if S not in _NC_CACHE:
        _NC_CACHE[S] = build_nc(S)
    return _NC_CACHE[S]


def make_in_maps(x, w_qkv, b_qkv, w_out, b_out):
    x = np.asarray(x, np.float32)
    w_qkv = np.asarray(w_qkv, np.float32)
    b_qkv = np.asarray(b_qkv, np.float32)
    w_out = np.asarray(w_out, np.float32)
    b_out = np.asarray(b_out, np.float32)
    B = x.shape[0]
    xt = np.ascontiguousarray(x.transpose(0, 2, 1)).astype(np.float16)
    wqkT = w_qkv[: 2 * C].T.astype(np.float16)          # [C, 2C]
    # [2C? -> t, p, ct, n] tiled layout: wqkTt[t, p, ct*128+n]
    wqkTt = np.ascontiguousarray(
        wqkT.reshape(NT_C, P, 2 * NT_C, P).transpose(2, 1, 0, 3).reshape(
            2 * NT_C, P, C
        )
    )
    wvT = np.ascontiguousarray(w_qkv[2 * C :].T).astype(np.float16)
    woT = np.ascontiguousarray(w_out.T).astype(np.float16)
    bqk = np.ascontiguousarray(b_qkv[: 2 * C])
    bv = np.ascontiguousarray(b_qkv[2 * C :])
    bo = np.ascontiguousarray(b_out)
    return [
        {
            "xt": xt[i],
            "wqkTt": wqkTt,
            "wvT": wvT,
            "woutT": woT,
            "bqk": bqk,
            "bv": bv,
            "bout": bo,
        }
        for i in range(B)
    ]


def kernel_with_results(x, w_qkv, b_qkv, w_out, b_out, attention_mask=None, **run_kw):
    from concourse.bass_utils import run_bass_kernel_spmd

    B, S, C_ = x.shape
    assert C_ == C
    nc = _get_nc(S)
    in_maps = make_in_maps(x, w_qkv, b_qkv, w_out, b_out)
    res = run_bass_kernel_spmd(nc, in_maps, core_ids=list(range(B)), **run_kw)
    out = np.stack([m["out"] for m in res.results], axis=0).astype(np.float32)
    return out, res


def kernel(x, w_qkv, b_qkv, w_out, b_out, attention_mask=None):
    out, _ = kernel_with_results(x, w_qkv, b_qkv, w_out, b_out, attention_mask)
    return out


# revision 21
# speedup vs baseline: 1.1682x; 1.0949x over previous
"""Causal self-attention Trainium2 kernel (B=8, S=1024, C=768, H=12).

Sharding: pure data-parallel over batch — core i computes batch i end-to-end.
No collectives. Weights are replicated to all 8 cores.

Software-pipelined schedule (v2): attention for q-block b runs interleaved
with projection work for later blocks and out-projection for earlier blocks,
so the Tensor engine never idles waiting on the Activation engine's exp.
Head PAIRS share one [128, 1024] 2-bank PSUM logits tile so a single exp
activation covers both heads (halves Act-engine instruction overhead).

Per-core math (batch b):
  xT        [C, S]   (host-transposed slice of x)
  Q,K       [c'=h*64+d, S] layout  (projection with feature dim on partitions)
  V(+ones)  [S, h, 65] layout      (natural layout + fused ones column)
  logits    [s_k, s_q] (transposed) -> exp on ScalarE -> P
  AV        psum[65, s_q] = [V_h | 1]^T P   (row 64 = softmax denominator)
  y         [c, S] layout, normalized via DMA-broadcast reciprocal row
  out       [S, C] via out-proj with y tiles as the stationary operand
"""

import sys
import types
from collections import deque

import numpy as np

import concourse.bass as bass
import concourse.mybir as mybir
import concourse.tile as tile
from concourse import bacc
from concourse.masks import make_upper_triangular


def _ensure_axon_hooks():
    """The container's `antenv` stub lacks `axon_hooks`, which
    run_bass_kernel_spmd imports when trace=True under axon. Provide it and
    register the NTFF profile hook so tracing works."""
    try:
        import antenv.axon_hooks  # noqa: F401

        return
    except ImportError:
        pass
    try:
        import antenv
    except ImportError:
        return
    mod = types.ModuleType("antenv.axon_hooks")
    _store = [None]
    mod.set_axon_ntff_profile_hook = lambda h: _store.__setitem__(0, h)
    mod.get_axon_ntff_profile_hook = lambda: _store[0]
    sys.modules["antenv.axon_hooks"] = mod
    antenv.axon_hooks = mod
    try:
        from trn_agent_boot.trn_boot import _ntff_profile_via_ctypes

        hook = _ntff_profile_via_ctypes("/opt/axon/libaxon_pjrt.so")
        mod.set_axon_ntff_profile_hook(hook)
    except Exception:
        pass


_ensure_axon_hooks()

P = 128
C = 768
H = 12
D = 64
NT_C = C // P          # 6 c-tiles
QB = 256               # q-block (matmul moving free dim)
F32 = mybir.dt.float32
F16 = mybir.dt.float16


def build_nc(S=1024):
    NT_S = S // P          # 8 s-tiles
    NB = S // QB           # 4 q-blocks

    nc = bacc.Bacc("TRN2", target_bir_lowering=False, debug=False)

    xt_d = nc.dram_tensor("xt", [C, S], F16, kind="ExternalInput")
    # wqkTt[t, p, ct*128+n] = wqkT[ct*128+p, t*128+n]: per-(t) tiles with
    # contiguous per-partition lines for efficient DMA.
    wqk_d = nc.dram_tensor("wqkTt", [2 * NT_C, P, C], F16, kind="ExternalInput")
    wv_d = nc.dram_tensor("wvT", [C, C], F16, kind="ExternalInput")
    wo_d = nc.dram_tensor("woutT", [C, C], F16, kind="ExternalInput")
    bqk_d = nc.dram_tensor("bqk", [2 * C], F32, kind="ExternalInput")
    bv_d = nc.dram_tensor("bv", [C], F32, kind="ExternalInput")
    bo_d = nc.dram_tensor("bout", [C], F32, kind="ExternalInput")
    out_d = nc.dram_tensor("out", [S, C], F32, kind="ExternalOutput")
    # scratch for the per-block reciprocal-denominator broadcast roundtrip
    dn_d = nc.dram_tensor("dn_scratch", [S // QB, H, QB], F16, kind="Internal")

    with tile.TileContext(nc) as tc:
        with (
            tc.tile_pool(name="const", bufs=1) as cpool,
            tc.tile_pool(name="big", bufs=1) as gpool,
            tc.tile_pool(name="ptile", bufs=4) as ppool,
            tc.tile_pool(name="evac", bufs=3) as epool,
            tc.tile_pool(name="denom", bufs=2) as gpool2,
            tc.tile_pool(name="recip", bufs=4) as rcpool,
            tc.tile_pool(name="bcast", bufs=2) as bpool,
            tc.tile_pool(name="proj_ps", bufs=2, space="PSUM") as proj_ps,
            tc.tile_pool(name="logit_ps", bufs=2, space="PSUM") as logit_ps,
            tc.tile_pool(name="av_ps", bufs=2, space="PSUM") as av_ps,
        ):
            # ---------------- constants ----------------
            # Fused diagonal-pair mask over a [s_k-pair, q-block] logits tile:
            # cols 0:128 tri, 128:256 keep, 256:384 zero, 384:512 tri.
            dmask = cpool.tile([P, 2 * QB], F16)
            make_upper_triangular(nc, dmask[:, 0:P], val=1.0, diag=True)
            nc.vector.memset(dmask[:, P:QB], 1.0)
            nc.vector.memset(dmask[:, QB : QB + P], 0.0)
            make_upper_triangular(nc, dmask[:, QB + P : 2 * QB], val=1.0, diag=True)
            dmask_r = dmask[:]

            bqk_sb = cpool.tile([P, 2 * NT_C], F32)
            nc.scalar.dma_start(bqk_sb[:], bqk_d[:].rearrange("(t p) -> p t", p=P))
            bv_bc = cpool.tile([P, C], F32)
            nc.scalar.dma_start(bv_bc[:], bv_d[:][None, :].to_broadcast((P, C)))
            bo_bc = cpool.tile([P, C], F32)
            nc.scalar.dma_start(bo_bc[:], bo_d[:][None, :].to_broadcast((P, C)))

            # ---------------- persistent SBUF tensors ----------------
            xt_sb = gpool.tile([P, NT_C, S], F16)
            qk_sb = gpool.tile([P, 2 * NT_C, S], F16)   # Q tiles 0..5, K 6..11
            vp_sb = gpool.tile([P, NT_S, H, D + 1], F16)  # [s, st, h, d|1]
            nc.vector.memset(vp_sb[:, :, :, D : D + 1], 1.0)
            y_sb = gpool.tile([P, NT_C, S], F16)

            wqk_sb = gpool.tile([P, 2 * NT_C, C], F16)  # [p, t, ct*128+n]
            wv_sb = gpool.tile([P, NT_C, C], F16)
            wo_sb = gpool.tile([P, NT_C, C], F16)

            xt_r = xt_d[:, :].rearrange("(ct p) s -> p ct s", p=P)
            wv_r = wv_d[:, :].rearrange("(ct p) n -> p ct n", p=P)
            wo_r = wo_d[:, :].rearrange("(ct p) n -> p ct n", p=P)

            # ---------------- input DMA schedule ----------------
            # sync queue: xt first half, wqk tiles (in consumption order),
            #             xt second half.
            HS = S // 2
            for ct in range(NT_C):
                nc.sync.dma_start(xt_sb[:, ct, 0:HS], xt_r[:, ct, 0:HS])
            for hp in range(NT_C):
                for t in (hp, NT_C + hp):
                    nc.sync.dma_start(wqk_sb[:, t, :], wqk_d[t, :, :])
            for ct in range(NT_C):
                nc.sync.dma_start(xt_sb[:, ct, HS:S], xt_r[:, ct, HS:S])
            # gpsimd (software DGE) queue: wv chunks then wo; the per-head
            # reciprocal broadcasts interleave after these in program order.
            for ct in range(NT_C):
                nc.gpsimd.dma_start(wv_sb[:, ct, 0:512], wv_r[:, ct, 0:512])
            for ct in range(NT_C):
                nc.gpsimd.dma_start(wv_sb[:, ct, 512:C], wv_r[:, ct, 512:C])
            for ct in range(NT_C):
                nc.gpsimd.dma_start(wo_sb[:, ct, :], wo_r[:, ct, :])

            # ---------------- work-item builders ----------------
            def qk_group(t, half):
                s0 = half * HS
                ps = proj_ps.tile([P, HS], F32, tag="proj")
                for ct in range(NT_C):
                    nc.tensor.matmul(
                        ps[:],
                        wqk_sb[:, t, ct * P : (ct + 1) * P],
                        xt_sb[:, ct, s0 : s0 + HS],
                        start=(ct == 0),
                        stop=(ct == NT_C - 1),
                    )
                nc.scalar.activation(
                    qk_sb[:, t, s0 : s0 + HS], ps[:],
                    mybir.ActivationFunctionType.Identity,
                    bias=bqk_sb[:, t : t + 1],
                )

            def v_group(st, ci):
                cs, cw = (0, 512) if ci == 0 else (512, 256)
                ps = proj_ps.tile([P, HS], F32, tag="proj")
                for ct in range(NT_C):
                    nc.tensor.matmul(
                        ps[:, :cw],
                        xt_sb[:, ct, st * P : (st + 1) * P],
                        wv_sb[:, ct, cs : cs + cw],
                        start=(ct == 0),
                        stop=(ct == NT_C - 1),
                    )
                nh = cw // D
                h0 = cs // D
                nc.vector.tensor_add(
                    vp_sb[:, st, h0 : h0 + nh, 0:D],
                    ps[:, :cw].rearrange("p (h d) -> p h d", d=D),
                    bv_bc[:, cs : cs + cw].rearrange("p (h d) -> p h d", d=D),
                )

            ot_tiles = {}

            def outp_group(st, ci):
                cs, cw = (0, 512) if ci == 0 else (512, 256)
                if ci == 0:
                    ot_tiles[st] = epool.tile([P, C], F32, tag="ot", name=f"ot_{st}")
                ot = ot_tiles[st]
                ps = proj_ps.tile([P, HS], F32, tag="proj")
                for ct in range(NT_C):
                    nc.tensor.matmul(
                        ps[:, :cw],
                        y_sb[:, ct, st * P : (st + 1) * P],
                        wo_sb[:, ct, cs : cs + cw],
                        start=(ct == 0),
                        stop=(ct == NT_C - 1),
                    )
                nc.vector.tensor_add(
                    ot[:, cs : cs + cw], ps[:, :cw], bo_bc[:, cs : cs + cw]
                )
                if ci == 1:
                    nc.sync.dma_start(out_d[st * P : (st + 1) * P, :], ot[:])

            # ---------------- attention (per head-pair) ----------------
            pending = deque()  # deferred y-normalization multiplies

            def flush_pending(k=None):
                n = len(pending) if k is None else min(k, len(pending))
                for _ in range(n):
                    pending.popleft()()

            dn_tiles = {}

            def attn_pair(b, hp, drain=None):
                flush_pending(3)
                if hp == 0:
                    dn_tiles[b] = gpool2.tile(
                        [H, QB], F32, tag="dn", name=f"dn_{b}"
                    )
                dn = dn_tiles[b]
                kt = NT_C + hp
                avs = [
                    av_ps.tile([D + 1, QB], F32, tag="av", name=f"av_{b}_{hp}_{hh}")
                    for hh in (0, 1)
                ]
                pts = []

                def av_mms(jp):
                    pt2 = pts[jp]
                    for hh in (0, 1):
                        h = 2 * hp + hh
                        for dj in (0, 1):
                            j = 2 * jp + dj
                            nc.tensor.matmul(
                                avs[hh][:],
                                vp_sb[:, j, h, :],
                                pt2[:, hh * 2 * QB + dj * QB : hh * 2 * QB + (dj + 1) * QB],
                                start=(j == 0),
                                stop=(j == 2 * b + 1),
                            )

                for jp in range(b + 1):
                    lg2 = logit_ps.tile([P, 4 * QB], F32, tag="lg")
                    for hh in (0, 1):
                        lo = hh * D
                        for dj in (0, 1):
                            j = 2 * jp + dj
                            nc.tensor.matmul(
                                lg2[:, hh * 2 * QB + dj * QB : hh * 2 * QB + (dj + 1) * QB],
                                qk_sb[lo : lo + D, kt, j * P : (j + 1) * P],
                                qk_sb[lo : lo + D, hp, b * QB : (b + 1) * QB],
                                start=True,
                                stop=True,
                                skip_group_check=True,
                            )
                    pt2 = ppool.tile([P, 4 * QB], F16, tag="pt")
                    nc.scalar.activation(
                        pt2[:], lg2[:],
                        mybir.ActivationFunctionType.Exp, scale=0.125,
                    )
                    if jp == b:  # diagonal pair: fused causal masking
                        for hh in (0, 1):
                            base = hh * 2 * QB
                            nc.vector.tensor_mul(
                                pt2[:, base : base + 2 * QB],
                                pt2[:, base : base + 2 * QB],
                                dmask_r,
                            )
                    pts.append(pt2)
                    if jp >= 1:
                        av_mms(jp - 1)
                    if drain is not None:
                        drain.step()
                av_mms(b)
                # stash denominator rows; evacuate unnormalized y
                for hh in (0, 1):
                    h = 2 * hp + hh
                    rc = rcpool.tile([1, QB], F32, tag="rc", name=f"rc_{b}_{h}")
                    nc.vector.tensor_copy(rc[:], avs[hh][D : D + 1, :])
                    nc.gpsimd.dma_start(dn[h : h + 1, :], rc[:])
                    lo2 = hh * D
                    nc.vector.tensor_copy(
                        y_sb[lo2 : lo2 + D, hp, b * QB : (b + 1) * QB],
                        avs[hh][0:D, :],
                    )

            def norm_block(b):
                """Batched reciprocal of the block's 12 denominator rows, one
                broadcast DMA through DRAM, then deferred in-place y scaling."""
                dn = dn_tiles[b]
                with nc.allow_low_precision(
                    reason="f32 reciprocal of softmax denominators"
                ):
                    nc.vector.reciprocal(dn[:], dn[:])
                nc.gpsimd.dma_start(dn_d[b], dn[:])  # casts f32 -> f16
                bc = bpool.tile([P, H, QB], F16, tag="bc", name=f"bc_{b}")
                nc.sync.dma_start(
                    bc[:], dn_d[b][None, :, :].to_broadcast((P, H, QB))
                )
                for h in range(H):
                    hp, hh = h // 2, h % 2
                    lo2 = hh * D

                    def _norm(bc=bc, h=h, hp=hp, lo2=lo2, b=b):
                        yv = y_sb[lo2 : lo2 + D, hp, b * QB : (b + 1) * QB]
                        nc.vector.tensor_mul(yv, yv, bc[lo2 : lo2 + D, h, :])

                    pending.append(_norm)

            # ---------------- filler drain ----------------
            class Drainer:
                """Issue filler work items spread across attention units.
                `late` items (out-projections, which depend on deferred
                normalization) are held until 55% of the stage has passed."""

                def __init__(self):
                    self.early = []
                    self.late = []
                    self.acc = 0.0
                    self.rate = 0.0
                    self.units = 1
                    self.u = 0

                def load(self, early, late, units):
                    self.early = list(early)
                    self.late = list(late)
                    self.acc = 0.0
                    self.units = max(units, 1)
                    self.u = 0
                    self.rate = (len(self.early) + len(self.late)) / self.units

                def _late_ok(self):
                    return self.u >= 0.55 * self.units

                def step(self):
                    self.u += 1
                    self.acc += self.rate
                    while self.acc >= 1.0 and (
                        self.early or (self.late and self._late_ok())
                    ):
                        src = self.early if self.early else self.late
                        src.pop(0)()
                        self.acc -= 1.0

                def flush(self):
                    for f in self.early + self.late:
                        f()
                    self.early = []
                    self.late = []

            drain = Drainer()

            # ---------------- prologue: proj(first half) + attn block 0 ----
            qk_group(0, 0)
            qk_group(NT_C + 0, 0)
            v_group(0, 0)
            v_group(1, 0)
            attn_pair(0, 0)
            for hp in range(1, NT_C):
                if hp == 4:
                    v_group(0, 1)
                    v_group(1, 1)
                qk_group(hp, 0)
                qk_group(NT_C + hp, 0)
                attn_pair(0, hp)
            norm_block(0)

            # ---------------- stages b = 1..3 ----------------
            fillers = {
                1: ([lambda t=t: qk_group(t, 1) for t in range(2 * NT_C)],
                    [lambda st=st, ci=ci: outp_group(st, ci)
                     for st in (0, 1) for ci in (0, 1)]),
                2: ([],
                    [lambda st=st, ci=ci: outp_group(st, ci)
                     for st in (2, 3) for ci in (0, 1)]),
                3: ([],
                    [lambda st=st, ci=ci: outp_group(st, ci)
                     for st in (4, 5) for ci in (0, 1)]),
            }
            for b in range(1, NB):
                for st in (2 * b, 2 * b + 1):
                    for ci in (0, 1):
                        v_group(st, ci)
                early, late = fillers[b]
                drain.load(early, late, units=NT_C * (b + 1))
                for hp in range(NT_C):
                    attn_pair(b, hp, drain)
                norm_block(b)
                drain.flush()
            flush_pending()
            for st in (6, 7):
                for ci in (0, 1):
                    outp_group(st, ci)

    nc.compile()
    return nc


_NC_CACHE = {}


def _get_nc(S):
    if S not in _NC_CACHE:
        _NC_CACHE[S] = build_nc(S)
    return _NC_CACHE[S]


def make_in_maps(x, w_qkv, b_qkv, w_out, b_out):
    x = np.asarray(x, np.float32)
    w_qkv = np.asarray(w_qkv, np.float32)
    b_qkv = np.asarray(b_qkv, np.float32)
    w_out = np.asarray(w_out, np.float32)
    b_out = np.asarray(b_out, np.float32)
    B = x.shape[0]
    xt = np.ascontiguousarray(x.transpose(0, 2, 1)).astype(np.float16)
    wqkT = w_qkv[: 2 * C].T.astype(np.float16)          # [C, 2C]
    # [2C? -> t, p, ct, n] tiled layout: wqkTt[t, p, ct*128+n]
    wqkTt = np.ascontiguousarray(
        wqkT.reshape(NT_C, P, 2 * NT_C, P).transpose(2, 1, 0, 3).reshape(
            2 * NT_C, P, C
        )
    )
    wvT = np.ascontiguousarray(w_qkv[2 * C :].T).astype(np.float16)
    woT = np.ascontiguousarray(w_out.T).astype(np.float16)
    bqk = np.ascontiguousarray(b_qkv[: 2 * C])
    bv = np.ascontiguousarray(b_qkv[2 * C :])
    bo = np.ascontiguousarray(b_out)
    return [
        {
            "xt": xt[i],
            "wqkTt": wqkTt,
            "wvT": wvT,
            "woutT": woT,
            "bqk": bqk,
            "bv": bv,
            "bout": bo,
        }
        for i in range(B)
    ]


def kernel_with_results(x, w_qkv, b_qkv, w_out, b_out, attention_mask=None, **run_kw):
    from concourse.bass_utils import run_bass_kernel_spmd

    B, S, C_ = x.shape
    assert C_ == C
    nc = _get_nc(S)
    in_maps = make_in_maps(x, w_qkv, b_qkv, w_out, b_out)
    res = run_bass_kernel_spmd(nc, in_maps, core_ids=list(range(B)), **run_kw)
    out = np.stack([m["out"] for m in res.results], axis=0).astype(np.float32)
    return out, res


def kernel(x, w_qkv, b_qkv, w_out, b_out, attention_mask=None):
    out, _ = kernel_with_results(x, w_qkv, b_qkv, w_out, b_out, attention_mask)
    return out
